# revision 19
# baseline (speedup 1.0000x reference)
"""CRF loss (shared-'I-' IE topology) for Trainium2, data-parallel over batch.

Math notes
----------
reference() loss = (num - den).sum() / num_tokens with, per batch row b:

  num_b = sum_valid_t lp[b,t,y_t] + lsm0[y_0]
          + sum_{t,t-1 both valid} lsmA[y_{t-1}, y_t] + lsmA[y_last, C]

  den_b: the 2-state forward scan
      alpha <- where(m_t, [a + L_t, a + lp0_t], alpha),  a = logaddexp(alpha0, alpha1)
    telescopes exactly (same logaddexp chain, reassociated):
      a_{k+1} = a_k + logaddexp(L_{t_k}, lp0_{t_k}) = a_k + z_{t_k}
      den_b   = sum_{valid t} z_t - z_{t_last} + L_{t_last}
    where z_t = logsumexp_c lp[b,t,:] and L_t = logsumexp_{c>=1} lp[b,t,c].

The memory-bound term is sum_valid z_t (touches all of log_probs, 100 MB).
The device kernel computes exactly that, sharded 8 batch rows per core:
exp on ACT (in-place), reduce over C on DVE, ln on ACT, mask by labels>=0,
per-partition accumulate. Everything else is O(B*T) label gathers and
O(C^2) tables, done on host in float64.
"""

import numpy as np
from contextlib import ExitStack

B, T, C = 64, 8192, 48
NCORES = 8
BP = B // NCORES          # batch rows per core
NCHUNK = 16               # device tiles per core
CHUNK_R = BP * T // NCHUNK   # (b,t) rows per tile
KT = CHUNK_R // 128       # rows per partition per tile
X_BUFS = 4
IGNORE = -100

_cache = {}


def _build_bass():
    import concourse.bacc as bacc
    import concourse.tile as tile
    from concourse import mybir

    nc = bacc.Bacc(name="crf_den")
    lp = nc.dram_tensor("lp", [BP * T, C], mybir.dt.float32, kind="ExternalInput")
    msk = nc.dram_tensor("msk", [BP * T], mybir.dt.int8, kind="ExternalInput")
    zacc = nc.dram_tensor("zacc", [128, NCHUNK], mybir.dt.float32, kind="ExternalOutput")

    X = mybir.AxisListType.X
    F32 = mybir.dt.float32
    with tile.TileContext(nc) as tc, ExitStack() as ctx:
        xp = ctx.enter_context(tc.tile_pool(name="x", bufs=X_BUFS))
        sp = ctx.enter_context(tc.tile_pool(name="s", bufs=3))
        lq = ctx.enter_context(tc.tile_pool(name="labs", bufs=1))
        apool = ctx.enter_context(tc.tile_pool(name="acc", bufs=1))

        acc = apool.tile([128, NCHUNK], F32)
        s_all = sp.tile([128, NCHUNK, KT], F32)
        for cb in range(NCHUNK):
            r0 = cb * CHUNK_R
            x = xp.tile([128, KT * C], F32)
            src = lp[r0 : r0 + CHUNK_R, :].rearrange("(p k) c -> p (k c)", p=128)
            nc.sync.dma_start(out=x, in_=src)
            nc.scalar.activation(out=x, in_=x, func=mybir.ActivationFunctionType.Exp)
            nc.vector.reduce_sum(
                s_all[:, cb, :], x.rearrange("p (k c) -> p k c", c=C), axis=X
            )
        # 0/1 mask bytes (off the critical path; traced late so the big
        # streaming DMAs go first): mt[p, n, k] = msk[n*128*KT + p*KT + k]
        mt = lq.tile([128, NCHUNK, KT], mybir.dt.int8)
        msrc = msk[:].rearrange("(n p k) -> p n k", n=NCHUNK, p=128)
        nc.sync.dma_start(out=mt, in_=msrc)
        mf = lq.tile([128, NCHUNK, KT], F32)
        nc.vector.tensor_copy(out=mf, in_=mt)  # int8 -> f32 cast
        # one tail pass: z = ln(s) * mask, per-chunk totals
        nc.scalar.activation(
            out=s_all, in_=s_all, func=mybir.ActivationFunctionType.Ln
        )
        nc.vector.tensor_mul(out=s_all, in0=s_all, in1=mf)
        nc.vector.reduce_sum(acc, s_all, axis=X)
        nc.sync.dma_start(out=zacc[:, :], in_=acc)
    nc.compile()
    return nc


def _get_nc():
    if "nc" not in _cache:
        _cache["nc"] = _build_bass()
    return _cache["nc"]


def _log_softmax(x, axis=-1):
    m = x.max(axis=axis, keepdims=True)
    return x - m - np.log(np.exp(x - m).sum(axis=axis, keepdims=True))


def _make_cached_runner(nc):
    """Cached jitted shard_map over the 8 cores — the same NEFF pipeline that
    run_bass_kernel_spmd's axon path uses (bass2jax._bass_exec_p), but reusable
    across kernel() calls so we don't re-trace/re-jit every invocation."""
    import jax
    from jax.sharding import Mesh, NamedSharding, PartitionSpec
    from jax.experimental.shard_map import shard_map
    from concourse import bass2jax, mybir

    bass2jax.install_neuronx_cc_hook()
    partition_name = nc.partition_id_tensor.name if nc.partition_id_tensor else None

    in_names, out_names, out_avals, zero_outs = [], [], [], []
    for alloc in nc.m.functions[0].allocations:
        if not isinstance(alloc, mybir.MemoryLocationSet):
            continue
        name = alloc.memorylocations[0].name
        if alloc.kind == "ExternalInput":
            if name != partition_name:
                in_names.append(name)
        elif alloc.kind == "ExternalOutput":
            out_names.append(name)
            shape = tuple(alloc.tensor_shape)
            dtype = mybir.dt.np(alloc.dtype)
            out_avals.append(jax.core.ShapedArray(shape, dtype))
            zero_outs.append(np.zeros(shape, dtype))
    n_params = len(in_names)
    all_names = list(in_names) + list(out_names)
    if partition_name is not None:
        all_names.append(partition_name)

    def _body(*args):
        operands = list(args)
        if partition_name is not None:
            operands.append(bass2jax.partition_id_tensor())
        return tuple(
            bass2jax._bass_exec_p.bind(
                *operands,
                out_avals=tuple(out_avals),
                in_names=tuple(all_names),
                out_names=tuple(out_names),
                lowering_input_output_aliases=(),
                sim_require_finite=True,
                sim_require_nnan=True,
                nc=nc,
            )
        )

    devices = jax.devices()[:NCORES]
    mesh = Mesh(np.asarray(devices), ("core",))
    in_specs = (PartitionSpec("core"),) * (n_params + len(out_names))
    out_specs = (PartitionSpec("core"),) * len(out_names)
    fn = jax.jit(
        shard_map(_body, mesh=mesh, in_specs=in_specs, out_specs=out_specs,
                  check_rep=False),
        keep_unused=True,
    )
    sharding = NamedSharding(mesh, PartitionSpec("core"))
    zeros_full = [
        np.zeros((NCORES * z.shape[0], *z.shape[1:]), z.dtype) for z in zero_outs
    ]

    def run(in_concat: dict):
        import jax as _jax

        args = [_jax.device_put(in_concat[n], sharding) for n in in_names]
        args += [_jax.device_put(z, sharding) for z in zeros_full]
        outs = fn(*args)
        return {
            name: np.asarray(outs[i]).reshape(NCORES, *out_avals[i].shape)
            for i, name in enumerate(out_names)
        }

    return run


def _run_device(lp, labels):
    """Per-core masked sum over t of z_t = logsumexp_c lp.  Returns (B,) f64."""
    mask8 = (labels != IGNORE).astype(np.int8)
    lp2 = np.ascontiguousarray(lp.reshape(B * T, C))
    msk = np.ascontiguousarray(mask8.reshape(B * T))

    zacc_per_core = None
    try:
        if "runner" not in _cache:
            _cache["runner"] = _make_cached_runner(_get_nc())
        outs = _cache["runner"]({"lp": lp2, "msk": msk})
        zacc_per_core = outs["zacc"]  # [NCORES, 128, NCHUNK]
    except Exception:
        _cache.pop("runner", None)
        from concourse.bass_utils import run_bass_kernel_spmd

        in_maps = []
        for ci in range(NCORES):
            in_maps.append(
                {
                    "lp": lp2[ci * BP * T : (ci + 1) * BP * T],
                    "msk": msk[ci * BP * T : (ci + 1) * BP * T],
                }
            )
        res = run_bass_kernel_spmd(_get_nc(), in_maps, core_ids=list(range(NCORES)))
        zacc_per_core = np.stack([r["zacc"] for r in res.results])

    # partition p of chunk cb holds rows [cb*CHUNK_R + p*KT, +KT) of the shard;
    # KT divides T, so each (p, cb) cell belongs to exactly one batch row.
    cell_b = (
        np.arange(NCHUNK)[None, :] * CHUNK_R + np.arange(128)[:, None] * KT
    ) // T  # [128, NCHUNK] local batch row
    zsum = np.zeros(B, np.float64)
    for ci in range(NCORES):
        np.add.at(zsum, ci * BP + cell_b, zacc_per_core[ci].astype(np.float64))
    return zsum


def kernel(**inputs):
    lp = np.ascontiguousarray(np.asarray(inputs["log_probs"], dtype=np.float32))
    labels_in = np.asarray(inputs["labels"])
    A_start = np.asarray(inputs["A_start"], dtype=np.float64)
    A_trans = np.asarray(inputs["A_trans"], dtype=np.float64)
    labels = labels_in.astype(np.int32).reshape(B, T)

    zsum = _run_device(lp, labels)

    mask = labels != IGNORE
    lengths = mask.sum(axis=1)
    y = np.where(mask, labels, 0).astype(np.intp)

    lsm0 = _log_softmax(A_start)
    lsmA = _log_softmax(A_trans, axis=-1)

    emis = np.take_along_axis(lp, y[..., None], axis=2)[..., 0].astype(np.float64)
    num_emis = (emis * mask).sum(axis=1)
    tmask = mask[:, 1:] & mask[:, :-1]
    num_trans = lsm0[y[:, 0]] + (lsmA[y[:, :-1], y[:, 1:]] * tmask).sum(axis=1)
    last_idx = np.clip(lengths - 1, 0, T - 1)
    y_last = y[np.arange(B), last_idx]
    num = num_emis + num_trans + lsmA[y_last, C]

    rows_last = lp[np.arange(B), last_idx, :].astype(np.float64)  # (B, 48)
    mx = rows_last.max(axis=1, keepdims=True)
    z_last = (mx + np.log(np.exp(rows_last - mx).sum(axis=1, keepdims=True)))[:, 0]
    r1 = rows_last[:, 1:]
    mx1 = r1.max(axis=1, keepdims=True)
    L_last = (mx1 + np.log(np.exp(r1 - mx1).sum(axis=1, keepdims=True)))[:, 0]
    den = np.where(lengths > 0, zsum - z_last + L_last, 0.0)

    loss = (num - den).sum() / lengths.sum()
    return np.float32(loss)
